# revision 1
# baseline (speedup 1.0000x reference)
"""FP8DynamicLinear Trainium2 kernel (8-core SPMD).

Reference semantics (nn_FP8DynamicLinear, native_fp8_support=False branch):
    xb      = bf16(x)
    amax    = max(|min(xb)|, |max(xb)|)            # per-tensor, fp32
    scale   = 448 / max(amax, 1e-12)
    qx      = e4m3fn(clip(xb * scale, +-448))
    a       = bf16(qx) * bf16(1/scale)             # bf16 [M, K]
    b       = bf16(weight) * bf16(weight_scale)    # bf16 [N, K]
    out     = a @ b.T + bias                       # bf16 [B, S, N]

Strategy (M-sharded: 1024 tokens per core, weight replicated):
  * Host: cast x -> bf16, transpose -> xT [K, M], slice M across 8 cores.
  * TRN fp8e4 tops out at +-240 (vs e4m3fn's 448) but the grids agree up to
    240, so quantizing at scale/2 (values <= 224) reproduces e4m3fn rounding
    exactly for all |v| >= 2^-6; the 2x is folded into the dequant factor.
  * fp8 fast path (weight values all on the e4m3fn grid, as produced by the
    reference's per-tensor weight quantization): host pre-casts wT*0.5 to TRN
    fp8 AND pre-tiles it into the exact SBUF consumption layout
    [nb][p][kt][i][n] so each n-block's weights arrive as ONE fully
    contiguous 2.1MB DMA (16KB per partition).  Device quantizes x to fp8
    and runs DoubleRow fp8 matmuls; psum * (4 * bf16(1/scale) * bf16(w_scale))
    + bias.
  * amax is per-core local (each core quantizes its own token slice with its
    own scale and dequantizes with the matching factor, so the result is
    self-consistent; no cross-core collective needed and no clipping by
    construction).
  * GEMM loop: n-block outermost (weights stream exactly once = 67MB/core),
    m-subtile inner with k-contiguous accumulation chains of 16 DoubleRow
    matmuls into a single PSUM bank; drains (scalar dequant + vector bias
    add + packed out DMA) overlap the next chain.
  * bf16 fallback (arbitrary weight): host computes b.T = bf16(wT *
    bf16(w_scale)); device builds a = bf16(qx * bf16(1/scale)) and runs a
    bf16 matmul via the composable matmul library; psum + bias.
"""

from contextlib import ExitStack

import numpy as np
import ml_dtypes

import concourse.bass as bass
import concourse.mybir as mybir
import concourse.tile as tile
from concourse.bass import ts, ds
from concourse.bass_utils import run_bass_kernel_spmd
from concourse.kernels.tile_matmul import (
    ShapeInfo,
    composable_matmul_tile_kernel,
    dma_from_dram_kxn,
    dma_to_dram_mxn,
)

P = 128
F8_MAX = 448.0
N_CORES = 8

# Problem shapes (hardcoded per spec)
B, S, K, N = 2, 4096, 4096, 16384
M = B * S              # 8192 tokens
M_C = M // N_CORES     # 1024 tokens per core
K_TILE = 512
K_SUB = K_TILE // P    # 4
K_TILES = K // K_TILE  # 8
NB = N // 512          # 32 n-blocks
DR_STEPS = K // 256    # 16 DoubleRow accumulation steps per chain
M_SUB = M_C // P       # 8 m-subtiles per core

_F32 = mybir.dt.float32
_BF16 = mybir.dt.bfloat16
_F8 = mybir.dt.float8e4

# ----------------------------------------------------------------------------
# Wait legalizer: this container's walrus rejects engine instructions with
# more than 1 inline sync-wait (and EventSemaphore with more than 2), but
# bass_rust nop-fusion fuses 2 waits + 1 update into one instruction.  Split
# the excess onto preceding InstEventSemaphore carriers on the same engine.
_EXEMPT = (
    "InstEventSemaphore",
    "InstUnconditionalBranch",
    "InstConditionalBranch",
    "InstCall",
    "InstRegisterMove",
)
_ES_CAP = 2


def _legalize_waits(nc, max_inline=1):
    n_split = 0
    for fn in nc.m.functions:
        for blk in fn.blocks:
            insts = blk.instructions
            i = 0
            while i < len(insts):
                inst = insts[i]
                si = inst.sync_info
                if (
                    si is not None
                    and len(si.on_wait) > max_inline
                    and inst.__class__.__name__ not in _EXEMPT
                ):
                    extra = list(si.on_wait[:-max_inline])
                    keep = list(si.on_wait[-max_inline:])
                    inst.sync_info = mybir.SyncInfo(
                        on_wait=keep, on_update=list(si.on_update)
                    )
                    carriers = [
                        mybir.InstEventSemaphore(
                            name=f"{inst.name}-wsplit{j}",
                            engine=inst.engine,
                            bass_nofuse=True,
                            sync_info=mybir.SyncInfo(
                                on_wait=extra[j : j + _ES_CAP], on_update=[]
                            ),
                        )
                        for j in range(0, len(extra), _ES_CAP)
                    ]
                    for kk, es in enumerate(carriers):
                        insts.insert(i + kk, es)
                    i += len(carriers)
                    n_split += 1
                i += 1
    return n_split


# ----------------------------------------------------------------------------
def build_nc_fp8(m_c=M_C, k=K, n=N, with_collective=True, gemm_passes=1,
                 repeats=1, psum_bufs=4, pair_nb=False, m_il=1):
    """fp8 fast path: custom n-outermost GEMM with k-contiguous PSUM chains.

    repeats>1 re-runs the whole phase A + GEMM body inside one NEFF for
    RPC-free device timing (t(R2)-t(R1))/(R2-R1)."""
    ks_tot = k // P            # 32
    k_tiles = k // K_TILE      # 8
    nb_cnt = n // 512          # 32
    dr_steps = k // 256        # 16
    m_sub = m_c // P           # 8

    nc = bass.Bass()
    xT = nc.dram_tensor("xT", [k, m_c], _BF16, kind="ExternalInput")
    # packed weights: [nb][p][dr][i][nn] so each nb is contiguous per partition
    wqp = nc.dram_tensor("wqp", [nb_cnt, P, dr_steps * 2 * 512], _F8,
                         kind="ExternalInput")
    bias_d = nc.dram_tensor("bias", [n], _BF16, kind="ExternalInput")
    wscale_d = nc.dram_tensor("wscale", [1], _F32, kind="ExternalInput")
    # packed out: [nb][m][p][nn]; host reassembles
    out_d = nc.dram_tensor("out", [nb_cnt, m_sub, P, 512], _BF16,
                           kind="ExternalOutput")

    xT3 = xT.rearrange("(o p) m -> p o m", p=P)  # k = o*128 + p

    with tile.TileContext(nc) as tc:
        with ExitStack() as ctx:
            const = ctx.enter_context(tc.tile_pool(name="const", bufs=1))
            dram = ctx.enter_context(tc.tile_pool(name="dram", bufs=1, space="DRAM"))

            # fp8 qx blocks, resident for the whole GEMM
            lhs_blocks = [
                const.tile([P, K_SUB, m_c], _F8, name=f"lhs{kt}", tag=f"lhs{kt}")
                for kt in range(k_tiles)
            ]
            bias_bc = const.tile([P, n], _BF16)
            scale_half = const.tile([P, 1], _F32)
            factor = const.tile([P, 1], _F32)  # 4*bf16(1/s)*bf16(ws)

            # GEMM-side pools allocated BEFORE phase A so SBUF space is
            # disjoint (space reuse would serialize w prefetch behind the
            # last quantize via a false pool-release dependency).
            wpool = ctx.enter_context(tc.tile_pool(name="wpool", bufs=4))
            out_pool = ctx.enter_context(tc.tile_pool(name="out_pool", bufs=4))
            psum = ctx.enter_context(
                tc.tile_pool(name="psum", bufs=psum_bufs, space="PSUM")
            )

            for _rep in range(repeats):
                # ---------------- Phase A: load bf16 x, absmax, quantize ---------
                with tc.tile_pool(name="phA", bufs=2) as pA:
                    xb = pA.tile([P, ks_tot, m_c], _BF16, bufs=1)
                    pmax_all = pA.tile([P, k_tiles], _F32, bufs=1)
                    for kt in range(k_tiles):
                        nc.sync.dma_start(
                            xb[:, ts(kt, K_SUB), :], xT3[:, ts(kt, K_SUB), :]
                        )
                        nc.vector.reduce_max(
                            pmax_all[:, ts(kt, 1)],
                            xb[:, ts(kt, K_SUB), :],
                            axis=mybir.AxisListType.XY,
                            apply_absolute_value=True,
                        )
                    pmax = pA.tile([P, 1], _F32, bufs=1)
                    nc.vector.reduce_max(
                        pmax[:], pmax_all[:], axis=mybir.AxisListType.X
                    )

                    # cross-partition max via DRAM bounce
                    pmax_dram = dram.tile([P], _F32)
                    nc.sync.dma_start(pmax_dram[:], pmax[:, 0])
                    pmax_row = pA.tile([1, P], _F32, bufs=1)
                    nc.sync.dma_start(pmax_row[:], pmax_dram[None, :])
                    amax_l = pA.tile([1, 1], _F32, bufs=1)
                    nc.vector.reduce_max(
                        amax_l[:], pmax_row[:], axis=mybir.AxisListType.X
                    )

                    # cross-core AllReduce(max): the reference quantizes x with
                    # the GLOBAL amax; using a different scale than the reference
                    # decorrelates the fp8 grids and ~13x-es the rel err.
                    amax = pA.tile([P, 1], _F32, bufs=1)
                    if with_collective:
                        cc_in = dram.tile([1], _F32)
                        cc_out = dram.tile([1], _F32, addr_space="Shared")
                        nc.sync.dma_start(cc_in[:], amax_l[0, :])
                        nc.gpsimd.collective_compute(
                            "AllReduce",
                            mybir.AluOpType.max,
                            ins=[cc_in[:]],
                            outs=[cc_out[:]],
                            replica_groups=[list(range(N_CORES))],
                        )
                        nc.sync.dma_start(
                            amax[:], cc_out[None, :].to_broadcast((P, 1)))
                    else:
                        amax_dram = dram.tile([1], _F32)
                        nc.sync.dma_start(amax_dram[:], amax_l[0, :])
                        nc.sync.dma_start(
                            amax[:], amax_dram[None, :].to_broadcast((P, 1)))

                    # scalar chain (replicated on all partitions):
                    # scale = 448/max(amax,1e-12); scale_half = scale/2
                    # factor = 4 * bf16(1/scale) * bf16(wscale)
                    nc.vector.tensor_scalar_max(amax[:], amax[:], 1e-12)
                    rcp = pA.tile([P, 1], _F32, bufs=1)
                    nc.vector.reciprocal(rcp[:], amax[:])
                    scale_t = pA.tile([P, 1], _F32, bufs=1)
                    nc.vector.tensor_scalar_mul(scale_t[:], rcp[:], F8_MAX)
                    nc.vector.tensor_scalar_mul(scale_half[:], scale_t[:], 0.5)
                    inv_s = pA.tile([P, 1], _F32, bufs=1)
                    nc.vector.reciprocal(inv_s[:], scale_t[:])
                    inv_b = pA.tile([P, 1], _BF16, bufs=1)
                    nc.vector.tensor_copy(inv_b[:], inv_s[:])
                    ws = pA.tile([P, 1], _F32, bufs=1)
                    nc.sync.dma_start(
                        ws[:], wscale_d[None, :].to_broadcast((P, 1)))
                    ws_b = pA.tile([P, 1], _BF16, bufs=1)
                    nc.vector.tensor_copy(ws_b[:], ws[:])
                    prod = pA.tile([P, 1], _F32, bufs=1)
                    nc.vector.tensor_tensor(
                        prod[:], inv_b[:], ws_b[:], mybir.AluOpType.mult
                    )
                    nc.vector.tensor_scalar_mul(factor[:], prod[:], 4.0)

                    # prefetch first w blocks now (x DMAs already queued ahead)
                    w_tiles = {}
                    n_pref = (W_PREFETCH + 1) if pair_nb else W_PREFETCH
                    for nb in range(min(n_pref, nb_cnt)):
                        _prefetch_w(nc, wpool, w_tiles, wqp, dr_steps, nb)

                    # quantize: fp8(xb * scale/2); both engines compute in fp32
                    # internally and RNE-cast to fp8 on the write.
                    for kt in range(k_tiles):
                        if kt % 2 == 0:
                            nc.scalar.activation(
                                lhs_blocks[kt][:],
                                xb[:, ts(kt, K_SUB), :],
                                mybir.ActivationFunctionType.Copy,
                                scale=scale_half[:],
                            )
                        else:
                            nc.vector.tensor_tensor(
                                lhs_blocks[kt][:],
                                xb[:, ts(kt, K_SUB), :],
                                scale_half[:, None].to_broadcast((P, K_SUB, m_c)),
                                mybir.AluOpType.mult,
                            )

                    # bias broadcast: emitted late so x/w loads win the DMA queues
                    nc.sync.dma_start(
                        bias_bc[:], bias_d[None, :].to_broadcast((P, n))
                    )

                # ---------------- Phase B: GEMM ----------------
                # n-block outermost; per (nb, m): 16 back-to-back DoubleRow
                # matmuls accumulate k=4096 into one PSUM bank; drain overlaps
                # the next chain.
                for _gp in range(gemm_passes):
                    if pair_nb:
                        _run_gemm_pass_pair(
                            nc, nb_cnt, m_sub, dr_steps, wpool, psum,
                            out_pool, w_tiles, wqp, lhs_blocks, bias_bc,
                            factor, out_d)
                    else:
                        _run_gemm_pass(
                            nc, nb_cnt, m_sub, dr_steps, wpool, psum,
                            out_pool, w_tiles, wqp, lhs_blocks, bias_bc,
                            factor, out_d, m_il=m_il)

    if pair_nb:
        n_elided = _elide_dup_ldweights(nc)
        print(f"pair_nb: elided {n_elided} ldweights")
    _legalize_waits(nc)
    return nc


def _elide_dup_ldweights(nc):
    """Delete the InstLdweights of a matmul whose stationary AP is identical
    to the immediately-preceding matmul's (PE stream pattern LDWa MMa LDWb
    MMb with LDWa==LDWb) — the weights are already in the array.  LDWb's
    sync waits/updates migrate onto MMb."""
    n = 0
    for fn in nc.m.functions:
        for blk in fn.blocks:
            insts = blk.instructions
            # positions of PE instructions in this block
            pe_idx = [i for i, it in enumerate(insts)
                      if getattr(it, "engine", None) == mybir.EngineType.PE]
            to_del = []
            for a, b, c, d in zip(pe_idx, pe_idx[1:], pe_idx[2:], pe_idx[3:]):
                ia, ib, ic, id_ = insts[a], insts[b], insts[c], insts[d]
                if not (isinstance(ia, mybir.InstLdweights)
                        and isinstance(ib, mybir.InstMatmult)
                        and isinstance(ic, mybir.InstLdweights)
                        and isinstance(id_, mybir.InstMatmult)):
                    continue
                wa, wc = ia.ins[0], ic.ins[0]
                if ((str(wa.memref), wa.offset, str(wa.ap))
                        != (str(wc.memref), wc.offset, str(wc.ap))):
                    continue
                if ia.perf_mode != ic.perf_mode:
                    continue
                si = ic.sync_info
                if si is not None and (si.on_wait or si.on_update):
                    mm_si = id_.sync_info or mybir.SyncInfo(
                        on_wait=[], on_update=[])
                    id_.sync_info = mybir.SyncInfo(
                        on_wait=list(si.on_wait) + list(mm_si.on_wait),
                        on_update=list(si.on_update) + list(mm_si.on_update),
                    )
                to_del.append(c)
            for i in reversed(to_del):
                del insts[i]
            n += len(to_del)
    return n


W_PREFETCH = 3  # wpool bufs must be >= W_PREFETCH + 1


def _prefetch_w(nc, wpool, w_tiles, wqp, dr_steps, nb):
    wt = wpool.tile([P, dr_steps, 2, 512], _F8, tag="w")
    nc.sync.dma_start(
        wt[:],
        wqp[nb].rearrange("p (d i nn) -> p d i nn", d=dr_steps, i=2),
    )
    w_tiles[nb] = wt


def _run_gemm_pass(nc, nb_cnt, m_sub, dr_steps, wpool, psum, out_pool,
                   w_tiles, wqp, lhs_blocks, bias_bc, factor, out_d,
                   m_il=1):
    if True:
        if True:
            for nb in range(nb_cnt):
                # emit future w DMAs BEFORE this block's out DMAs so the
                # prefetch stream never head-blocks behind drain-gated
                # writes in the qSP FIFO
                for pf in range(nb, min(nb + W_PREFETCH + 1, nb_cnt)):
                    if pf not in w_tiles:
                        _prefetch_w(nc, wpool, w_tiles, wqp, dr_steps, pf)
                wt = w_tiles.pop(nb)
                for m0 in range(0, m_sub, m_il):
                    ms = list(range(m0, min(m0 + m_il, m_sub)))
                    pts = [psum.tile([P, 512], _F32, tag="ps",
                                     name=f"ps_il{mi}")
                           for mi in range(len(ms))]
                    for dr in range(dr_steps):
                        for mi, m in enumerate(ms):
                            lhsT = lhs_blocks[dr // 2][
                                :, ds((dr % 2) * 2, 2), ds(m * P, P)
                            ]
                            nc.tensor.matmul(
                                pts[mi][:],
                                lhsT,
                                wt[:, dr],
                                start=(dr == 0),
                                stop=(dr == dr_steps - 1),
                                perf_mode=mybir.MatmulPerfMode.DoubleRow,
                            )
                    for mi, m in enumerate(ms):
                        st = out_pool.tile([P, 512], _BF16, tag="o")
                        nc.scalar.activation(
                            st[:], pts[mi][:],
                            mybir.ActivationFunctionType.Copy,
                            scale=factor[:],
                        )
                        nc.vector.tensor_tensor(
                            st[:], st[:],
                            bias_bc[:, ds(nb * 512, 512)],
                            mybir.AluOpType.add,
                        )
                        nc.sync.dma_start(out_d[nb, m], st[:])


def _run_gemm_pass_pair(nc, nb_cnt, m_sub, dr_steps, wpool, psum, out_pool,
                        w_tiles, wqp, lhs_blocks, bias_bc, factor, out_d):
    """nb-pair interleave: consecutive matmuls share the stationary lhsT
    (two n-chunks per k-step), enabling ldweights elision on the second."""
    assert nb_cnt % 2 == 0
    for j in range(nb_cnt // 2):
        nbA, nbB = 2 * j, 2 * j + 1
        for pf in range(nbA, min(nbA + 4, nb_cnt)):
            if pf not in w_tiles:
                _prefetch_w(nc, wpool, w_tiles, wqp, dr_steps, pf)
        wtA = w_tiles.pop(nbA)
        wtB = w_tiles.pop(nbB)
        for m in range(m_sub):
            ptA = psum.tile([P, 512], _F32, tag="ps")
            ptB = psum.tile([P, 512], _F32, tag="ps")
            for dr in range(dr_steps):
                lhsT = lhs_blocks[dr // 2][
                    :, ds((dr % 2) * 2, 2), ds(m * P, P)
                ]
                for pt, wt in ((ptA, wtA), (ptB, wtB)):
                    nc.tensor.matmul(
                        pt[:],
                        lhsT,
                        wt[:, dr],
                        start=(dr == 0),
                        stop=(dr == dr_steps - 1),
                        perf_mode=mybir.MatmulPerfMode.DoubleRow,
                    )
            for nb, pt in ((nbA, ptA), (nbB, ptB)):
                st = out_pool.tile([P, 512], _BF16, tag="o")
                nc.scalar.activation(
                    st[:], pt[:],
                    mybir.ActivationFunctionType.Copy,
                    scale=factor[:],
                )
                nc.vector.tensor_tensor(
                    st[:], st[:],
                    bias_bc[:, ds(nb * 512, 512)],
                    mybir.AluOpType.add,
                )
                nc.sync.dma_start(out_d[nb, m], st[:])


# ----------------------------------------------------------------------------
def build_nc_bf16(m_c=M_C, k=K, n=N):
    """Generic fallback: bf16 GEMM via the composable matmul library."""
    ks_tot = k // P
    k_tiles = k // K_TILE

    nc = bass.Bass()
    xT = nc.dram_tensor("xT", [k, m_c], _F32, kind="ExternalInput")
    wq = nc.dram_tensor("wq", [k, n], _BF16, kind="ExternalInput")
    bias_d = nc.dram_tensor("bias", [n], _BF16, kind="ExternalInput")
    wscale_d = nc.dram_tensor("wscale", [1], _F32, kind="ExternalInput")
    out_d = nc.dram_tensor("out", [m_c, n], _BF16, kind="ExternalOutput")

    xT3 = xT.rearrange("(o p) m -> p o m", p=P)

    with tile.TileContext(nc) as tc:
        with ExitStack() as ctx:
            const = ctx.enter_context(tc.tile_pool(name="const", bufs=1))
            dram = ctx.enter_context(tc.tile_pool(name="dram", bufs=1, space="DRAM"))

            lhs_blocks = [
                const.tile([P, K_SUB, m_c], _BF16, name=f"lhs{kt}", tag=f"lhs{kt}")
                for kt in range(k_tiles)
            ]
            bias_bc = const.tile([P, n], _BF16)
            scale_half = const.tile([P, 1], _F32)
            factor = const.tile([P, 1], _F32)  # 2*bf16(1/s)

            kxn_pool = ctx.enter_context(
                tc.tile_pool(name="kxn_pool", bufs=k_tiles + 1)
            )
            out_pool = ctx.enter_context(tc.tile_pool(name="out_pool", bufs=3))
            qtmp_pool = ctx.enter_context(tc.tile_pool(name="qtmp", bufs=2))

            with tc.tile_pool(name="phA", bufs=2) as pA:
                pmax_all = pA.tile([P, k_tiles], _F32, bufs=1)
                for kt in range(k_tiles):
                    xs = pA.tile([P, K_SUB, m_c], _F32, tag="xstage")
                    nc.sync.dma_start(xs[:], xT3[:, ts(kt, K_SUB), :])
                    xbt = pA.tile([P, K_SUB, m_c], _BF16, tag="xbt")
                    nc.scalar.activation(
                        xbt[:], xs[:],
                        mybir.ActivationFunctionType.Copy,
                    )
                    nc.vector.reduce_max(
                        pmax_all[:, ts(kt, 1)],
                        xbt[:],
                        axis=mybir.AxisListType.XY,
                        apply_absolute_value=True,
                    )
                pmax = pA.tile([P, 1], _F32, bufs=1)
                nc.vector.reduce_max(
                    pmax[:], pmax_all[:], axis=mybir.AxisListType.X
                )

                pmax_dram = dram.tile([P], _F32)
                nc.sync.dma_start(pmax_dram[:], pmax[:, 0])
                pmax_row = pA.tile([1, P], _F32, bufs=1)
                nc.sync.dma_start(pmax_row[:], pmax_dram[None, :])
                amax_l = pA.tile([1, 1], _F32, bufs=1)
                nc.vector.reduce_max(
                    amax_l[:], pmax_row[:], axis=mybir.AxisListType.X
                )
                # cross-core AllReduce(max): must match the reference's
                # global amax exactly (see fp8 path comment)
                amax = pA.tile([P, 1], _F32, bufs=1)
                cc_in = dram.tile([1], _F32)
                cc_out = dram.tile([1], _F32, addr_space="Shared")
                nc.sync.dma_start(cc_in[:], amax_l[0, :])
                nc.gpsimd.collective_compute(
                    "AllReduce",
                    mybir.AluOpType.max,
                    ins=[cc_in[:]],
                    outs=[cc_out[:]],
                    replica_groups=[list(range(N_CORES))],
                )
                nc.sync.dma_start(
                    amax[:], cc_out[None, :].to_broadcast((P, 1))
                )

                nc.vector.tensor_scalar_max(amax[:], amax[:], 1e-12)
                rcp = pA.tile([P, 1], _F32, bufs=1)
                nc.vector.reciprocal(rcp[:], amax[:])
                scale_t = pA.tile([P, 1], _F32, bufs=1)
                nc.vector.tensor_scalar_mul(scale_t[:], rcp[:], F8_MAX)
                nc.vector.tensor_scalar_mul(scale_half[:], scale_t[:], 0.5)
                inv_s = pA.tile([P, 1], _F32, bufs=1)
                nc.vector.reciprocal(inv_s[:], scale_t[:])
                inv_b = pA.tile([P, 1], _BF16, bufs=1)
                nc.vector.tensor_copy(inv_b[:], inv_s[:])
                nc.vector.tensor_scalar_mul(factor[:], inv_b[:], 2.0)

                for kt in range(k_tiles):
                    xs2 = pA.tile([P, K_SUB, m_c], _F32, tag="xstage")
                    nc.sync.dma_start(xs2[:], xT3[:, ts(kt, K_SUB), :])
                    xbt2 = pA.tile([P, K_SUB, m_c], _BF16, tag="xbt")
                    nc.scalar.activation(
                        xbt2[:], xs2[:],
                        mybir.ActivationFunctionType.Copy,
                    )
                    qt = qtmp_pool.tile([P, K_SUB, m_c], _F8, tag="qtmp")
                    nc.vector.tensor_tensor(
                        qt[:],
                        xbt2[:],
                        scale_half[:, None].to_broadcast((P, K_SUB, m_c)),
                        mybir.AluOpType.mult,
                    )
                    nc.scalar.activation(
                        lhs_blocks[kt][:],
                        qt[:],
                        mybir.ActivationFunctionType.Copy,
                        scale=factor[:],
                    )

                nc.sync.dma_start(
                    bias_bc[:], bias_d[None, :].to_broadcast((P, n))
                )

            kxm_shape = ShapeInfo(pdims=((P, ks_tot),), fdims=(m_c,))

            def kxm_producer(nc_, md):
                assert md.k_batch_idx == 0 and md.m_batch_idx == 0
                assert md.k_subtiles == K_SUB
                return lhs_blocks[md.k_tile_idx][
                    :, :, ds(md.m_tile_idx * md.m_tile, md.m_tile)
                ]

            kxn_producer, kxn_shape = dma_from_dram_kxn(kxn_pool, wq[:])

            def mxn_producer(nc_, md):
                prod_tile = out_pool.tile(
                    [min(P, md.m_tile), md.m_subtiles, md.n_tile],
                    _BF16,
                    name="mxn_out",
                    tag="mxn_out",
                )
                return prod_tile

            def reducer(nc_, psum, sbuf, md):
                start = (md.n_tile_idx * md.n_tile
                         + md.n_subtile_idx * md.n_subtile)
                sz = md.n_slice_size
                nc_.vector.tensor_tensor(
                    sbuf[:, :, :sz],
                    psum[:, :sz],
                    bias_bc[: psum.shape[0], ds(start, sz)],
                    mybir.AluOpType.add,
                )

            composable_matmul_tile_kernel(
                tc=tc,
                kxm_shape=kxm_shape,
                kxn_shape=kxn_shape,
                output_type=_BF16,
                kxm_producer=kxm_producer,
                kxn_producer=kxn_producer,
                mxn_consumer=dma_to_dram_mxn(out_d[:]),
                mxn_subtile_reducer=reducer,
                mxn_subtile_producer=mxn_producer,
                MATMUL_FREE_DIM=512,
                MAX_TILE_SIZE=512,
                MAX_K_TILE_SIZE=K_TILE,
                cache_tiles=True,
                temps_n_bufs=3,
                psum_n_bufs=2,
            )

    _legalize_waits(nc)
    return nc


# ----------------------------------------------------------------------------
_NC_CACHE = {}


def _get_nc(m_c=M_C, k=K, n=N, mode="fp8"):
    key = (m_c, k, n, mode)
    if key not in _NC_CACHE:
        if mode == "fp8":
            _NC_CACHE[key] = build_nc_fp8(m_c, k, n)
        else:
            _NC_CACHE[key] = build_nc_bf16(m_c, k, n)
    return _NC_CACHE[key]


def _weight_is_fp8_grid(w):
    """True if the *0.5 -> TRN fp8e4 cast reproduces the weights accurately
    enough for the fp8 fast path.

    The reference's pre-quantized weights are on the e4m3fn grid; halved,
    nearly all land exactly on the TRN fp8e4 grid.  The exception is e4m3fn
    subnormals (|w| < 2^-8): their halves fall below TRN's subnormal
    granularity and round.  Those are tiny and rare, so instead of exact
    equality we bound the relative RMS error introduced by the cast."""
    half = w.astype(np.float32) * np.float32(0.5)
    rt = half.astype(ml_dtypes.float8_e4m3).astype(np.float32)
    err = rt - half
    denom = float(np.sum(half * half))
    if denom == 0.0:
        return bool(np.all(err == 0))
    rel_rms = float(np.sqrt(np.sum(err * err) / denom))
    # mismatches must be both small in aggregate and individually tiny
    return rel_rms < 1e-3 and float(np.abs(err).max()) <= 2.0 ** -9


def prepare_in_maps(x, weight, weight_scale, bias, m_c=M_C, n_cores=N_CORES,
                    mode="fp8"):
    m = x.shape[0] * x.shape[1] if x.ndim == 3 else x.shape[0]
    k = x.shape[-1]
    ws = np.asarray(weight_scale, dtype=np.float32).reshape(1)
    bias_np = np.asarray(bias).astype(ml_dtypes.bfloat16)
    if mode == "fp8":
        # bf16 cast on host (the reference's first step), then transpose
        xb = np.asarray(x, dtype=np.float32).reshape(m, k).astype(
            ml_dtypes.bfloat16)
        xT = xb.T  # [k, m] view
        # pack weights: wqp[nb][p][dr][i][nn] with
        # w row index = dr*256 + i*128 + p, col = nb*512 + nn
        wq = (np.asarray(weight, dtype=np.float32).T
              * np.float32(0.5)).astype(ml_dtypes.float8_e4m3)  # [k, n]
        wqp = np.ascontiguousarray(
            wq.reshape(DR_STEPS, 2, P, NB, 512)      # [dr][i][p][nb][nn]
            .transpose(3, 2, 0, 1, 4)                # [nb][p][dr][i][nn]
        ).reshape(NB, P, DR_STEPS * 2 * 512)
        in_maps = []
        for c in range(n_cores):
            in_maps.append(
                {
                    "xT": np.ascontiguousarray(xT[:, c * m_c : (c + 1) * m_c]),
                    "wqp": wqp,
                    "bias": bias_np,
                    "wscale": ws,
                }
            )
        return in_maps
    else:
        x2 = np.ascontiguousarray(
            np.asarray(x, dtype=np.float32).reshape(m, k))
        xT = x2.T
        wT = np.ascontiguousarray(np.asarray(weight, dtype=np.float32).T)
        ws_b = np.float32(ws[0].astype(ml_dtypes.bfloat16))
        wq = (wT * ws_b).astype(ml_dtypes.bfloat16)
        in_maps = []
        for c in range(n_cores):
            in_maps.append(
                {
                    "xT": np.ascontiguousarray(xT[:, c * m_c : (c + 1) * m_c]),
                    "wq": wq,
                    "bias": bias_np,
                    "wscale": ws,
                }
            )
        return in_maps


def unpack_out(arr, mode="fp8"):
    """Per-core device output -> [m_c, N]."""
    if mode == "fp8":
        # arr: [NB, M_SUB, P, 512] -> [m, n]
        return np.ascontiguousarray(
            arr.transpose(1, 2, 0, 3).reshape(M_C, N))
    return arr.reshape(M_C, N)


def kernel(x, weight, weight_scale, bias):
    w = np.asarray(weight, dtype=np.float32)
    mode = "fp8" if _weight_is_fp8_grid(w) else "bf16"
    nc = _get_nc(mode=mode)
    in_maps = prepare_in_maps(x, w, weight_scale, bias, mode=mode)
    res = run_bass_kernel_spmd(nc, in_maps, list(range(N_CORES)))
    out = np.concatenate(
        [unpack_out(res.results[c]["out"], mode) for c in range(N_CORES)],
        axis=0)
    return out.reshape(B, S, N)

